# revision 10
# baseline (speedup 1.0000x reference)
"""GCN layer (gather -> mean-aggregate -> linear) on 8 Trainium2 cores.

Strategy (SPMD, no collectives), v3:
  - Nodes row-sharded: core c owns dst [c*S, (c+1)*S), S = N/8 = 1250.
  - Edges bucketed by dst-owner on host into a dense per-core adjacency
    count matrix A[src_slab, dst_local] (fp8e4m3, exact small ints). The
    segment sum is a dense GEMM x STATIONARY / A MOVING:
    sumsT[feat, dst] += xq_k.T @ A_k per 128-src slab k.
  - Packed input stream: per slab the DRAM row holds
    [xq bf16 256B | A 1250B] contiguously; SBUF views are bitcast
    slices. Each multi-slab group is column-split across the two HWDGE
    queues (sync: bytes 0:768, scalar: 768:1506) so slab arrival is
    smooth and both queues stay byte-balanced.
  - dst columns split in 4 bank-aligned chunks {384,256,384,226}, each
    accumulating in its own PSUM bank over all slabs. Chunk pipelines
    are SKEWED (lags 0/5/10/15 slabs) so chunks finish staggered and
    each chunk's output GEMM + scaled psum copies + output DMA overlap
    the remaining phase-B matmuls; only the last chunk's tail is serial.
  - Mean division folded into the psum->SBUF output copy as a
    per-partition (= per dst node) f32 scale; zero-in-degree nodes get a
    host self-edge. Copies split scalar/vector to halve latency.
  - x ships bf16 (gate 2e-2; bf16 adds ~0.2% error), +b applied on host.
"""

import numpy as np

CORES = 8
TRACE = False           # set by test harness to print HW exec time
_cache = {}


def _build_program(N, F, FO, R, RA):
    from concourse import bacc, tile
    from concourse.bass import mybir

    F32 = mybir.dt.float32
    BF16 = mybir.dt.bfloat16
    FP8 = mybir.dt.float8e4
    KT = (N + 127) // 128          # src slabs
    NLAST = N - 128 * (KT - 1)     # real rows in the last slab
    NT = R // 128                  # node tiles per core
    ROWB = 2 * F + RA              # packed bytes per slab per partition
    # chunk -> (dst col range, psum col base, node tiles)
    CHUNKS = [(0, 512), (512, 1024), (1024, RA)]
    PBASE = [0, 512, 1024]
    CTILES = [(0, 4), (4, 8), (8, NT)]
    LAGS = [0, 6, 12]
    NCH = len(CHUNKS)
    nc = bacc.Bacc(None)

    Dd = nc.dram_tensor("D", [128, KT * ROWB], mybir.dt.uint8,
                        kind="ExternalInput")
    Wd = nc.dram_tensor("W", [F, FO], BF16, kind="ExternalInput")
    invdd = nc.dram_tensor("invd", [128, NT], F32, kind="ExternalInput")
    outd = nc.dram_tensor("out", [R, FO], BF16, kind="ExternalOutput")

    # PSUM bank map (2KB banks): banks 0..3 chunk accumulators,
    # banks 4..6 output-GEMM (rotating), bank 7 PE warm-up.
    psall = nc.alloc_psum_tensor("psall", [128, 4096], F32)

    with tile.TileContext(nc) as tc:
        with tc.tile_pool(name="main", bufs=1) as pool:
            D_sb = pool.tile([128, KT, ROWB], mybir.dt.uint8, name="D_sb")
            wt_sb = pool.tile([128, FO], BF16, name="wt_sb")
            invd_sb = pool.tile([128, NT], F32, name="invd_sb")
            hT_sb = pool.tile([128, 4, 512], BF16, name="hT_sb")
            out_sb = pool.tile([128, NT, FO], BF16, name="out_sb")
            warm = pool.tile([128, 256], BF16, name="warm")
            qwarm = pool.tile([1, 128], mybir.dt.uint8, name="qwarm")

            Dd3 = Dd[:].rearrange("p (k b) -> p k b", b=ROWB)

            def xq_ap(k, pk=128):
                return D_sb[:pk, k, 0 : 2 * F].bitcast(BF16)

            def a_ap(k, d0, d1, pk=128):
                return D_sb[:pk, k, 2 * F + d0 : 2 * F + d1].bitcast(FP8)

            # tiny first transfers get the HWDGE queues spinning while the
            # real first group's descriptors are still being generated
            nc.sync.dma_start(qwarm[:, 0:64], Dd[:1, 0:64])
            nc.scalar.dma_start(qwarm[:, 64:128], Dd[:1, 64:128])

            # ---- input stream: contiguous whole-row slab groups, small at
            # first for a quick start, alternating between the queues ----
            bounds = [0, 1, 2, 3, 4, 6, 8, 12, 16, 20, 24, 28, 32]
            while bounds[-1] + 8 < KT - 1:
                bounds.append(bounds[-1] + 8)
            bounds.append(KT - 1)
            bounds.append(KT)
            for gi, (k0, k1) in enumerate(zip(bounds, bounds[1:])):
                pk = NLAST if k0 == KT - 1 else 128
                eng = nc.sync if gi % 2 == 0 else nc.scalar
                eng.dma_start(D_sb[:pk, k0:k1, :], Dd3[:pk, k0:k1, :])
                if gi == 5:
                    nc.scalar.dma_start(wt_sb[:], Wd[:])
                    nc.sync.dma_start(invd_sb[:], invdd[:])

            nc.vector.memset(warm[:], 0.0)

            # PE warm-up during the first DMA wait (HAM clock ramp)
            def warm_mm():
                nc.tensor.matmul(
                    psall[:16, 3584:3840], warm[:, 0:16], warm[:],
                    start=True, stop=True, skip_group_check=True,
                )

            for _w in range(6):
                warm_mm()

            # ---- skewed phase B + pipelined phase C ----
            pending = []
            out_ready = [0] * NT

            def emit_chunk_c(ci):
                d0, d1 = CHUNKS[ci]
                t0, t1 = CTILES[ci]
                cw = d1 - d0

                def cp_hT(ci=ci, cw=cw):
                    pb = PBASE[ci]
                    half = min((cw // 2 + 63) // 64 * 64, cw)
                    nc.scalar.copy(hT_sb[:, ci, 0:half],
                                   psall[:, pb : pb + half])
                    if cw > half:
                        nc.vector.tensor_copy(hT_sb[:, ci, half:cw],
                                              psall[:, pb + half : pb + cw])
                pending.append(cp_hT)

                for t in range(t0, t1):
                    def gemm(t=t, ci=ci, t0=t0):
                        pb = 1536 + (t % 3) * 512
                        nc.tensor.matmul(
                            psall[:, pb : pb + FO],
                            hT_sb[:, ci, 128 * (t - t0) : 128 * (t - t0) + 128],
                            wt_sb[:], start=True, stop=True,
                            skip_group_check=True,
                        )
                        # psum -> out rows scaled by f32 inv-degree; whole
                        # tiles alternate engines (scalar has a ~0.5us fixed
                        # cost per op, so within-tile splits pay it twice)
                        sc = invd_sb[:, t : t + 1]
                        if t % 3 == 2:
                            nc.scalar.activation(
                                out_sb[:, t, 0:FO], psall[:, pb : pb + FO],
                                mybir.ActivationFunctionType.Copy, scale=sc)
                        else:
                            nc.vector.tensor_scalar_mul(
                                out_sb[:, t, 0:FO], psall[:, pb : pb + FO],
                                sc)
                        out_ready[t] = 1
                        if t % 2 == 1 and out_ready[t - 1]:
                            tp = t - 1
                            if t < NT - 1:
                                dst = outd[128 * tp : 128 * tp + 256, :] \
                                    .rearrange("(a p) f -> p a f", p=128)
                                eng = nc.sync if (tp // 2) % 2 == 0 \
                                    else nc.scalar
                                eng.dma_start(dst, out_sb[:, tp : tp + 2, :])
                            else:  # split last pair: shorter drain
                                nc.scalar.dma_start(
                                    outd[128 * tp : 128 * tp + 128, :],
                                    out_sb[:, tp, :])
                                nc.sync.dma_start(
                                    outd[128 * tp + 128 : 128 * tp + 256, :],
                                    out_sb[:, tp + 1, :])
                    pending.append(gemm)

            for s in range(KT + LAGS[-1]):
                for ci in range(NCH):
                    k = s - LAGS[ci]
                    if not (0 <= k < KT):
                        continue
                    d0, d1 = CHUNKS[ci]
                    pk = 128 if k < KT - 1 else NLAST
                    nc.tensor.matmul(
                        psall[:, PBASE[ci] : PBASE[ci] + (d1 - d0)],
                        xq_ap(k, pk), a_ap(k, d0, d1, pk),
                        start=(k == 0), stop=(k == KT - 1),
                        skip_group_check=False,
                    )
                    if k == KT - 1:
                        emit_chunk_c(ci)
                if 1 <= s < 9:
                    warm_mm()   # keep the PE clock hot while slots are thin
                if pending:
                    pending.pop(0)()
            while pending:
                pending.pop(0)()

    nc.compile()
    return nc


def _shard_inputs(x32, src, dst, W32, b32, n_cores):
    import ml_dtypes

    BF = ml_dtypes.bfloat16
    F8 = ml_dtypes.float8_e4m3
    N, F = x32.shape
    S = (N + n_cores - 1) // n_cores
    NT = (S + 127) // 128
    R = NT * 128
    RA = S                      # real dst columns in A (psum pads to R)
    KT = (N + 127) // 128
    ROWB = 2 * F + RA

    deg = np.bincount(dst, minlength=N).astype(np.float32)
    zd = np.where(deg == 0)[0].astype(np.int64)

    # x in [partition=src%128, slab=src//128, feat] layout, bf16 bytes
    xf = np.zeros((KT * 128, F), np.float32)
    xf[:N] = x32
    xq = np.ascontiguousarray(
        xf.reshape(KT, 128, F).transpose(1, 0, 2)).astype(BF)  # [128,KT,F]
    xq_bytes = xq.view(np.uint8).reshape(128, KT, 2 * F)

    Wq = np.ascontiguousarray(W32).astype(BF)

    in_maps = []
    for c in range(n_cores):
        lo = c * S
        hi = min(N, lo + S)
        sel = (dst >= lo) & (dst < hi)
        s = src[sel]
        d = dst[sel] - lo
        zs = zd[(zd >= lo) & (zd < hi)]
        if len(zs):  # self-edges so zero-in-degree nodes keep their input
            s = np.concatenate([s, zs])
            d = np.concatenate([d, zs - lo])
        idx = (s % 128) * (KT * RA) + (s // 128) * RA + d
        cnt = np.bincount(idx, minlength=128 * KT * RA)
        assert cnt.max() <= 16, "edge multiplicity too large for fp8e4m3"
        A = cnt.astype(np.float32).reshape(128, KT, RA).astype(F8)

        D = np.empty((128, KT, ROWB), np.uint8)
        D[:, :, : 2 * F] = xq_bytes
        D[:, :, 2 * F :] = A.view(np.uint8)
        D = np.ascontiguousarray(D.reshape(128, KT * ROWB))

        degc = np.ones(R, np.float32)
        degc[: hi - lo] = np.maximum(deg[lo:hi], 1.0)
        invd = np.ascontiguousarray(
            (1.0 / degc).reshape(NT, 128).T)           # [128, NT]

        in_maps.append({"D": D, "W": Wq, "invd": invd})
    return in_maps, R, RA


def _install_ntff_shim():
    """antenv.axon_hooks shim so trace=True can NTFF-profile in this env."""
    import contextlib
    import ctypes
    import sys
    import types

    if "antenv.axon_hooks" in sys.modules:
        return
    so_path = "/opt/axon/libaxon_pjrt.so"
    try:
        lib = ctypes.CDLL(so_path)
        lib.axon_start_nrt_profile.argtypes = [
            ctypes.POINTER(ctypes.c_int64), ctypes.c_size_t]
        lib.axon_start_nrt_profile.restype = ctypes.c_int64
        lib.axon_stop_nrt_profile.argtypes = [ctypes.c_char_p]
        lib.axon_stop_nrt_profile.restype = ctypes.c_int64
    except Exception:
        return

    @contextlib.contextmanager
    def _hook(output_dir, device_ids):
        import jax

        jax.devices()
        if device_ids:
            ids = (ctypes.c_int64 * len(device_ids))(*device_ids)
            rc = lib.axon_start_nrt_profile(ids, len(device_ids))
        else:
            rc = lib.axon_start_nrt_profile(None, 0)
        if rc != 0:
            raise RuntimeError(f"axon_start_nrt_profile rc={rc}")
        try:
            yield
        finally:
            lib.axon_stop_nrt_profile(str(output_dir).encode())

    mod = types.ModuleType("antenv.axon_hooks")
    mod.set_axon_ntff_profile_hook = lambda h: None
    mod.get_axon_ntff_profile_hook = lambda: _hook
    sys.modules["antenv.axon_hooks"] = mod


def kernel(x, src, dst, W, b):
    from concourse import bass_utils

    x32 = np.ascontiguousarray(np.asarray(x), dtype=np.float32)
    W32 = np.ascontiguousarray(np.asarray(W), dtype=np.float32)
    b32 = np.ascontiguousarray(np.asarray(b), dtype=np.float32)
    src = np.asarray(src).astype(np.int64)
    dst = np.asarray(dst).astype(np.int64)
    N, F = x32.shape
    FO = W32.shape[1]
    S = (N + CORES - 1) // CORES

    in_maps, R, RA = _shard_inputs(x32, src, dst, W32, b32, CORES)

    key = (N, F, FO, R, RA)
    if key not in _cache:
        _cache[key] = _build_program(N, F, FO, R, RA)
    nc = _cache[key]

    if TRACE:
        _install_ntff_shim()

    import time as _time

    last_err = None
    for _attempt in range(3):
        try:
            res = bass_utils.run_bass_kernel_spmd(
                nc, in_maps, core_ids=list(range(CORES)), trace=TRACE
            )
            break
        except Exception as e:  # transient device errors; pause lets a
            last_err = e        # wedged core recover before the retry
            _time.sleep(3)
    else:
        raise last_err

    if TRACE and res.exec_time_ns is not None:
        print("HW exec time:", res.exec_time_ns, "ns")

    outs = [np.asarray(res.results[c]["out"]).astype(np.float32).reshape(R, FO)
            for c in range(CORES)]
    full = np.concatenate([o[:S] for o in outs], axis=0)[:N]
    return (full + b32.reshape(1, -1)).astype(np.float32)


# revision 12
# speedup vs baseline: 1.0287x; 1.0287x over previous
"""GCN layer (gather -> mean-aggregate -> linear) on 8 Trainium2 cores.

Strategy (SPMD, no collectives), v3:
  - Nodes row-sharded: core c owns dst [c*S, (c+1)*S), S = N/8 = 1250.
  - Edges bucketed by dst-owner on host into a dense per-core adjacency
    count matrix A[src_slab, dst_local] (fp8e4m3, exact small ints). The
    segment sum is a dense GEMM x STATIONARY / A MOVING:
    sumsT[feat, dst] += xq_k.T @ A_k per 128-src slab k.
  - Packed input stream: per slab the DRAM row holds
    [xq bf16 256B | A 1250B] contiguously; SBUF views are bitcast
    slices. Each multi-slab group is column-split across the two HWDGE
    queues (sync: bytes 0:768, scalar: 768:1506) so slab arrival is
    smooth and both queues stay byte-balanced.
  - dst columns split in 4 bank-aligned chunks {384,256,384,226}, each
    accumulating in its own PSUM bank over all slabs. Chunk pipelines
    are SKEWED (lags 0/5/10/15 slabs) so chunks finish staggered and
    each chunk's output GEMM + scaled psum copies + output DMA overlap
    the remaining phase-B matmuls; only the last chunk's tail is serial.
  - Mean division folded into the psum->SBUF output copy as a
    per-partition (= per dst node) f32 scale; zero-in-degree nodes get a
    host self-edge. Copies split scalar/vector to halve latency.
  - x ships bf16 (gate 2e-2; bf16 adds ~0.2% error), +b applied on host.
"""

import numpy as np

CORES = 8
TRACE = False           # set by test harness to print HW exec time
_cache = {}


def _build_program(N, F, FO, R, RA):
    from concourse import bacc, tile
    from concourse.bass import mybir

    F32 = mybir.dt.float32
    BF16 = mybir.dt.bfloat16
    FP8 = mybir.dt.float8e4
    KT = (N + 127) // 128          # src slabs
    NLAST = N - 128 * (KT - 1)     # real rows in the last slab
    NT = R // 128                  # node tiles per core
    ROWB = 2 * F + RA              # packed bytes per slab per partition
    # chunk -> (dst col range, psum col base, node tiles)
    CHUNKS = [(0, 512), (512, 1024), (1024, RA)]
    PBASE = [0, 512, 1024]
    CTILES = [(0, 4), (4, 8), (8, NT)]
    LAGS = [0, 6, 12]
    NCH = len(CHUNKS)
    nc = bacc.Bacc(None)

    Dd = nc.dram_tensor("D", [128, KT * ROWB], mybir.dt.uint8,
                        kind="ExternalInput")
    Wd = nc.dram_tensor("W", [F, FO], BF16, kind="ExternalInput")
    invdd = nc.dram_tensor("invd", [128, NT], F32, kind="ExternalInput")
    outd = nc.dram_tensor("out", [R, FO], BF16, kind="ExternalOutput")

    # PSUM bank map (2KB banks): banks 0..3 chunk accumulators,
    # banks 4..6 output-GEMM (rotating), bank 7 PE warm-up.
    psall = nc.alloc_psum_tensor("psall", [128, 4096], F32)

    with tile.TileContext(nc) as tc:
        with tc.tile_pool(name="main", bufs=1) as pool:
            D_sb = pool.tile([128, KT, ROWB], mybir.dt.uint8, name="D_sb")
            wt_sb = pool.tile([128, FO], BF16, name="wt_sb")
            invd_sb = pool.tile([128, NT], F32, name="invd_sb")
            hT_sb = pool.tile([128, 4, 512], BF16, name="hT_sb")
            out_sb = pool.tile([128, NT, FO], BF16, name="out_sb")
            warm = pool.tile([128, 256], BF16, name="warm")
            qwarm = pool.tile([1, 128], mybir.dt.uint8, name="qwarm")

            Dd3 = Dd[:].rearrange("p (k b) -> p k b", b=ROWB)

            def xq_ap(k, pk=128):
                return D_sb[:pk, k, 0 : 2 * F].bitcast(BF16)

            def a_ap(k, d0, d1, pk=128):
                return D_sb[:pk, k, 2 * F + d0 : 2 * F + d1].bitcast(FP8)

            # tiny first transfers get the HWDGE queues spinning while the
            # real first group's descriptors are still being generated
            nc.sync.dma_start(qwarm[:, 0:64], Dd[:1, 0:64])
            nc.scalar.dma_start(qwarm[:, 64:128], Dd[:1, 64:128])

            # ---- input stream: contiguous whole-row slab groups, small at
            # first for a quick start, alternating between the queues ----
            bounds = [0, 1, 2, 3, 4, 6, 8, 12, 16]
            while bounds[-1] + 4 < KT - 1:
                bounds.append(bounds[-1] + 4)
            bounds.append(KT - 1)
            bounds.append(KT)
            for gi, (k0, k1) in enumerate(zip(bounds, bounds[1:])):
                pk = NLAST if k0 == KT - 1 else 128
                eng = nc.sync if gi % 2 == 0 else nc.scalar
                eng.dma_start(D_sb[:pk, k0:k1, :], Dd3[:pk, k0:k1, :])
                if gi == 5:
                    nc.scalar.dma_start(wt_sb[:], Wd[:])
                    nc.sync.dma_start(invd_sb[:], invdd[:])

            nc.vector.memset(warm[:], 0.0)

            # PE warm-up during the first DMA wait (HAM clock ramp)
            def warm_mm():
                nc.tensor.matmul(
                    psall[:16, 3584:3840], warm[:, 0:16], warm[:],
                    start=True, stop=True, skip_group_check=True,
                )

            for _w in range(6):
                warm_mm()

            # ---- skewed phase B + pipelined phase C ----
            pending = []
            out_ready = [0] * NT

            def emit_chunk_c(ci):
                d0, d1 = CHUNKS[ci]
                t0, t1 = CTILES[ci]
                cw = d1 - d0

                def cp_hT(ci=ci, cw=cw):
                    pb = PBASE[ci]
                    half = min((cw // 2 + 63) // 64 * 64, cw)
                    nc.scalar.copy(hT_sb[:, ci, 0:half],
                                   psall[:, pb : pb + half])
                    if cw > half:
                        nc.vector.tensor_copy(hT_sb[:, ci, half:cw],
                                              psall[:, pb + half : pb + cw])
                pending.append(cp_hT)

                for t in range(t0, t1):
                    def gemm(t=t, ci=ci, t0=t0):
                        pb = 1536 + (t % 3) * 512
                        nc.tensor.matmul(
                            psall[:, pb : pb + FO],
                            hT_sb[:, ci, 128 * (t - t0) : 128 * (t - t0) + 128],
                            wt_sb[:], start=True, stop=True,
                            skip_group_check=True,
                        )
                        # psum -> out rows scaled by f32 inv-degree; whole
                        # tiles alternate engines (scalar has a ~0.5us fixed
                        # cost per op, so within-tile splits pay it twice)
                        sc = invd_sb[:, t : t + 1]
                        if t % 3 == 2:
                            nc.scalar.activation(
                                out_sb[:, t, 0:FO], psall[:, pb : pb + FO],
                                mybir.ActivationFunctionType.Copy, scale=sc)
                        else:
                            nc.vector.tensor_scalar_mul(
                                out_sb[:, t, 0:FO], psall[:, pb : pb + FO],
                                sc)
                        out_ready[t] = 1
                        if t % 2 == 1 and out_ready[t - 1]:
                            tp = t - 1
                            if t < NT - 1:
                                dst = outd[128 * tp : 128 * tp + 256, :] \
                                    .rearrange("(a p) f -> p a f", p=128)
                                eng = nc.sync if (tp // 2) % 2 == 0 \
                                    else nc.scalar
                                eng.dma_start(dst, out_sb[:, tp : tp + 2, :])
                            else:  # split last pair: shorter drain
                                nc.scalar.dma_start(
                                    outd[128 * tp : 128 * tp + 128, :],
                                    out_sb[:, tp, :])
                                nc.sync.dma_start(
                                    outd[128 * tp + 128 : 128 * tp + 256, :],
                                    out_sb[:, tp + 1, :])
                    pending.append(gemm)

            for s in range(KT + LAGS[-1]):
                for ci in range(NCH):
                    k = s - LAGS[ci]
                    if not (0 <= k < KT):
                        continue
                    d0, d1 = CHUNKS[ci]
                    pk = 128 if k < KT - 1 else NLAST
                    nc.tensor.matmul(
                        psall[:, PBASE[ci] : PBASE[ci] + (d1 - d0)],
                        xq_ap(k, pk), a_ap(k, d0, d1, pk),
                        start=(k == 0), stop=(k == KT - 1),
                        skip_group_check=False,
                    )
                    if k == KT - 1:
                        emit_chunk_c(ci)
                if 1 <= s < 13:
                    warm_mm()   # keep the PE clock hot while slots are thin
                    warm_mm()
                if pending:
                    pending.pop(0)()
            while pending:
                pending.pop(0)()

    nc.compile()
    return nc


def _shard_inputs(x32, src, dst, W32, b32, n_cores):
    import ml_dtypes

    BF = ml_dtypes.bfloat16
    F8 = ml_dtypes.float8_e4m3
    N, F = x32.shape
    S = (N + n_cores - 1) // n_cores
    NT = (S + 127) // 128
    R = NT * 128
    RA = S                      # real dst columns in A (psum pads to R)
    KT = (N + 127) // 128
    ROWB = 2 * F + RA

    deg = np.bincount(dst, minlength=N).astype(np.float32)
    zd = np.where(deg == 0)[0].astype(np.int64)

    # x in [partition=src%128, slab=src//128, feat] layout, bf16 bytes
    xf = np.zeros((KT * 128, F), np.float32)
    xf[:N] = x32
    xq = np.ascontiguousarray(
        xf.reshape(KT, 128, F).transpose(1, 0, 2)).astype(BF)  # [128,KT,F]
    xq_bytes = xq.view(np.uint8).reshape(128, KT, 2 * F)

    Wq = np.ascontiguousarray(W32).astype(BF)

    in_maps = []
    for c in range(n_cores):
        lo = c * S
        hi = min(N, lo + S)
        sel = (dst >= lo) & (dst < hi)
        s = src[sel]
        d = dst[sel] - lo
        zs = zd[(zd >= lo) & (zd < hi)]
        if len(zs):  # self-edges so zero-in-degree nodes keep their input
            s = np.concatenate([s, zs])
            d = np.concatenate([d, zs - lo])
        idx = (s % 128) * (KT * RA) + (s // 128) * RA + d
        cnt = np.bincount(idx, minlength=128 * KT * RA)
        assert cnt.max() <= 16, "edge multiplicity too large for fp8e4m3"
        A = cnt.astype(np.float32).reshape(128, KT, RA).astype(F8)

        D = np.empty((128, KT, ROWB), np.uint8)
        D[:, :, : 2 * F] = xq_bytes
        D[:, :, 2 * F :] = A.view(np.uint8)
        D = np.ascontiguousarray(D.reshape(128, KT * ROWB))

        degc = np.ones(R, np.float32)
        degc[: hi - lo] = np.maximum(deg[lo:hi], 1.0)
        invd = np.ascontiguousarray(
            (1.0 / degc).reshape(NT, 128).T)           # [128, NT]

        in_maps.append({"D": D, "W": Wq, "invd": invd})
    return in_maps, R, RA


def _install_ntff_shim():
    """antenv.axon_hooks shim so trace=True can NTFF-profile in this env."""
    import contextlib
    import ctypes
    import sys
    import types

    if "antenv.axon_hooks" in sys.modules:
        return
    so_path = "/opt/axon/libaxon_pjrt.so"
    try:
        lib = ctypes.CDLL(so_path)
        lib.axon_start_nrt_profile.argtypes = [
            ctypes.POINTER(ctypes.c_int64), ctypes.c_size_t]
        lib.axon_start_nrt_profile.restype = ctypes.c_int64
        lib.axon_stop_nrt_profile.argtypes = [ctypes.c_char_p]
        lib.axon_stop_nrt_profile.restype = ctypes.c_int64
    except Exception:
        return

    @contextlib.contextmanager
    def _hook(output_dir, device_ids):
        import jax

        jax.devices()
        if device_ids:
            ids = (ctypes.c_int64 * len(device_ids))(*device_ids)
            rc = lib.axon_start_nrt_profile(ids, len(device_ids))
        else:
            rc = lib.axon_start_nrt_profile(None, 0)
        if rc != 0:
            raise RuntimeError(f"axon_start_nrt_profile rc={rc}")
        try:
            yield
        finally:
            lib.axon_stop_nrt_profile(str(output_dir).encode())

    mod = types.ModuleType("antenv.axon_hooks")
    mod.set_axon_ntff_profile_hook = lambda h: None
    mod.get_axon_ntff_profile_hook = lambda: _hook
    sys.modules["antenv.axon_hooks"] = mod


def kernel(x, src, dst, W, b):
    from concourse import bass_utils

    x32 = np.ascontiguousarray(np.asarray(x), dtype=np.float32)
    W32 = np.ascontiguousarray(np.asarray(W), dtype=np.float32)
    b32 = np.ascontiguousarray(np.asarray(b), dtype=np.float32)
    src = np.asarray(src).astype(np.int64)
    dst = np.asarray(dst).astype(np.int64)
    N, F = x32.shape
    FO = W32.shape[1]
    S = (N + CORES - 1) // CORES

    in_maps, R, RA = _shard_inputs(x32, src, dst, W32, b32, CORES)

    key = (N, F, FO, R, RA)
    if key not in _cache:
        _cache[key] = _build_program(N, F, FO, R, RA)
    nc = _cache[key]

    if TRACE:
        _install_ntff_shim()

    import time as _time

    last_err = None
    for _attempt in range(3):
        try:
            res = bass_utils.run_bass_kernel_spmd(
                nc, in_maps, core_ids=list(range(CORES)), trace=TRACE
            )
            break
        except Exception as e:  # transient device errors; pause lets a
            last_err = e        # wedged core recover before the retry
            _time.sleep(3)
    else:
        raise last_err

    if TRACE and res.exec_time_ns is not None:
        print("HW exec time:", res.exec_time_ns, "ns")

    outs = [np.asarray(res.results[c]["out"]).astype(np.float32).reshape(R, FO)
            for c in range(CORES)]
    full = np.concatenate([o[:S] for o in outs], axis=0)[:N]
    return (full + b32.reshape(1, -1)).astype(np.float32)
